# revision 7
# baseline (speedup 1.0000x reference)
"""Weighted-BCE loss kernel for Trainium2 (8 NeuronCores, SPMD data-parallel).

Reference math (torch-style BCELoss with class-balancing weights):
    n   = len(x), s = sum(gt)
    w0  = n / (2*(n-s)),  w1 = n / (2*s)
    L1  = max(log(x),     -100)
    L0  = max(log1p(-x),  -100)
    loss = mean( where(gt==0, w0, w1) * -(gt*L1 + (1-gt)*L0) )

The weights depend only on the GLOBAL positive count s, so the loss
decomposes into 4 global sums computed shard-locally:
    A = sum(gt * L1),  B = sum(gt * L0),  C = sum(L0),  s = sum(gt)
    loss = -( A/(2s) + (C-B)/(2(n-s)) )

Each core processes a 1/8 shard laid out [128 partitions, 16384 free].
Engine placement (v3):
  - All input DMAs stream through the single SP HWDGE ring, interleaved
    x_i, gt_i in consumption order, so tile 0 lands at full bandwidth.
  - ScalarE (ACT): lnx = Ln(x) and ln1 = Ln(1-x), both emitted as
    bf16.  A -inf from Ln(0) is safe: lnx has no accum_out and the DVE
    max(-100) clamp restores exact reference semantics.  ln1 is always
    finite (x < 1) so its accum_out produces C for free.  2 ops/tile.
  - VectorE (DVE): gtb = max(gt,gt) cast to bf16 (1x, int32 input)
    whose accum_out produces S; then A/B as all-bf16
    scalar_tensor_tensor ops which run in the DVE 2x 16-bit mode.
    accum_out scalars stay f32 (exempt from the 2x dtype rule).
  - bf16 rounding of the log values adds ~0.4% random per-element error
    which cancels in the 2M-element sums (~4e-6 relative on A/B/C).
Host gathers the [128, 4*ntiles] partials from all 8 cores and finishes
the (tiny) all-reduce + final scalar arithmetic in float64.
"""

import numpy as np
from contextlib import ExitStack

import concourse.bass as bass
import concourse.bacc as bacc
import concourse.mybir as mybir
import concourse.tile as tile
from concourse.alu_op_type import AluOpType
from concourse.bass_utils import run_bass_kernel_spmd

N_TOTAL = 16777216
N_CORES = 8
PER_CORE = N_TOTAL // N_CORES   # 2097152
P = 128
FD = PER_CORE // P              # 16384 free elements per partition
TILE_SIZES = [2048] * 7 + [1024] * 2
assert sum(TILE_SIZES) == FD
NT = len(TILE_SIZES)
LOG_CLAMP = -100.0

# Optional instrumentation knobs for a driver script (harness never sets them).
TRACE = False
LAST_RESULTS = None

_NC_CACHE = None


def _build():
    f32 = mybir.dt.float32
    i32 = mybir.dt.int32
    bf16 = mybir.dt.bfloat16
    Ln = mybir.ActivationFunctionType.Ln

    nc = bacc.Bacc("TRN2")
    x_in = nc.declare_dram_parameter("x", [P, FD], f32, isOutput=False)
    g_in = nc.declare_dram_parameter("gt", [P, FD], i32, isOutput=False)
    # one packed output: columns [A | B | C | S], NT each
    out_all = nc.declare_dram_parameter("out_all", [P, 4 * NT], f32, isOutput=True)

    with tile.TileContext(nc) as tc, ExitStack() as ctx:
        xp = ctx.enter_context(tc.tile_pool(name="xp", bufs=3))
        gp = ctx.enter_context(tc.tile_pool(name="gp", bufs=3))
        gbp = ctx.enter_context(tc.tile_pool(name="gbp", bufs=2))
        lp = ctx.enter_context(tc.tile_pool(name="lp", bufs=2))
        jp = ctx.enter_context(tc.tile_pool(name="jp", bufs=1))
        accp = ctx.enter_context(tc.tile_pool(name="accp", bufs=1))

        acc = accp.tile([P, 4 * NT], f32)

        def col(group, i):
            j = group * NT + i
            return acc[:, j : j + 1]

        off = 0
        for i, tfd in enumerate(TILE_SIZES):
            sl = slice(off, off + tfd)
            off += tfd
            xt = xp.tile([P, tfd], f32, tag="xt")
            gt_t = gp.tile([P, tfd], i32, tag="gt")
            # single SP ring, consumption order: tile i's data lands before
            # tile i+1's, each transfer at full aggregate queue bandwidth
            nc.sync.dma_start(xt[:], x_in[:, sl])
            nc.sync.dma_start(gt_t[:], g_in[:, sl])

            lnx = lp.tile([P, tfd], bf16, tag="lnx")
            ln1 = lp.tile([P, tfd], bf16, tag="ln1")
            nc.scalar.activation(lnx[:], xt[:], Ln)
            nc.scalar.activation(
                ln1[:], xt[:], Ln, bias=1.0, scale=-1.0,
                accum_out=col(2, i),
            )

            # int32 -> bf16 cast of gt (1x: int32 input); accum gives S free
            gtb = gbp.tile([P, tfd], bf16, tag="gtb")
            nc.vector.scalar_tensor_tensor(
                gtb[:], gt_t[:], 0.0, gt_t[:],
                AluOpType.max, AluOpType.max,
                accum_out=col(3, i),
            )
            # A/B: all-bf16 operands -> DVE 2x 16-bit mode
            junk = jp.tile([P, tfd], bf16, tag="junk")
            nc.vector.scalar_tensor_tensor(
                junk[:], lnx[:], LOG_CLAMP, gtb[:],
                AluOpType.max, AluOpType.mult,
                accum_out=col(0, i),
            )
            junk2 = jp.tile([P, tfd], bf16, tag="junk")
            nc.vector.scalar_tensor_tensor(
                junk2[:], ln1[:], LOG_CLAMP, gtb[:],
                AluOpType.max, AluOpType.mult,
                accum_out=col(1, i),
            )

        nc.sync.dma_start(out_all[:], acc[:])

    nc.compile()
    return nc


def get_nc():
    global _NC_CACHE
    if _NC_CACHE is None:
        _NC_CACHE = _build()
    return _NC_CACHE


def make_in_maps(x, gt):
    x = np.ascontiguousarray(np.asarray(x, dtype=np.float32).reshape(-1))
    gt = np.ascontiguousarray(np.asarray(gt, dtype=np.int32).reshape(-1))
    assert x.shape == (N_TOTAL,) and gt.shape == (N_TOTAL,)
    in_maps = []
    for c in range(N_CORES):
        sl = slice(c * PER_CORE, (c + 1) * PER_CORE)
        in_maps.append({
            "x": x[sl].reshape(P, FD),
            "gt": gt[sl].reshape(P, FD),
        })
    return in_maps


def combine(results):
    """All-reduce the per-core partial sums and finish the loss formula."""
    A = B = C = S = 0.0
    for r in results:
        o = r["out_all"].astype(np.float64)
        A += o[:, 0 * NT : 1 * NT].sum()
        B += o[:, 1 * NT : 2 * NT].sum()
        C += o[:, 2 * NT : 3 * NT].sum()
        S += o[:, 3 * NT : 4 * NT].sum()
    n = float(N_TOTAL)
    result = -(A / (2.0 * S) + (C - B) / (2.0 * (n - S)))
    return np.array(result, dtype=np.float32)


def kernel(x, gt):
    global LAST_RESULTS
    nc = get_nc()
    in_maps = make_in_maps(x, gt)
    br = run_bass_kernel_spmd(nc, in_maps, list(range(N_CORES)))
    LAST_RESULTS = br
    return combine(br.results)


# revision 8
# speedup vs baseline: 1.1676x; 1.1676x over previous
"""Weighted-BCE loss kernel for Trainium2 (8 NeuronCores, SPMD data-parallel).

Reference math (torch-style BCELoss with class-balancing weights):
    n   = len(x), s = sum(gt)
    w0  = n / (2*(n-s)),  w1 = n / (2*s)
    L1  = max(log(x),     -100)
    L0  = max(log1p(-x),  -100)
    loss = mean( where(gt==0, w0, w1) * -(gt*L1 + (1-gt)*L0) )

The weights depend only on the GLOBAL positive count s, so the loss
decomposes into 4 global sums computed shard-locally:
    A = sum(gt * L1),  B = sum(gt * L0),  C = sum(L0),  s = sum(gt)
    loss = -( A/(2s) + (C-B)/(2(n-s)) )

Each core processes a 1/8 shard laid out [128 partitions, 16384 free].
Engine placement (v4 — every engine under the ~39us DMA roofline):
  - All input DMAs stream through the single SP HWDGE ring, interleaved
    x_i, gt_i in consumption order; bufs=5 tile pools let the SP ring
    post transfers well ahead so the 16 HW DMA queues never gap.
  - ScalarE (ACT): Ln(x) and Ln(1-x) (free affine scale=-1, bias=1),
    emitted as bf16; the second op's accum_out produces C for free.
    Plus a short Copy+accum over the 4x-folded gt to finish S.
  - VectorE (DVE): two fused scalar_tensor_tensor ops, each doing
    clamp(max, -100) + multiply-by-gt + row-reduce in one instruction
    (A and B).  gt (int32) is consumed directly as the in1 operand.
    A -inf from Ln(0) is clamped to exactly -100 here, matching torch.
  - GpSimd (Pool): two add-folds gt[:, :h]+gt[:, h:] shrink gt 4x so
    ACT's S pass only touches tfd/4 columns.
  - bf16 rounding of the log values adds ~0.4% random per-element error
    which cancels in the 2M-element sums (~5e-7 observed end to end).
Host gathers the [128, 4*ntiles] partials from all 8 cores and finishes
the (tiny) all-reduce + final scalar arithmetic in float64.
"""

import numpy as np
from contextlib import ExitStack

import concourse.bass as bass
import concourse.bacc as bacc
import concourse.mybir as mybir
import concourse.tile as tile
from concourse.alu_op_type import AluOpType
from concourse.bass_utils import run_bass_kernel_spmd

N_TOTAL = 16777216
N_CORES = 8
PER_CORE = N_TOTAL // N_CORES   # 2097152
P = 128
FD = PER_CORE // P              # 16384 free elements per partition
TILE_SIZES = [2048] * 7 + [1024] * 2
assert sum(TILE_SIZES) == FD
NT = len(TILE_SIZES)
LOG_CLAMP = -100.0

# Optional instrumentation knobs for a driver script (harness never sets them).
TRACE = False
LAST_RESULTS = None

_NC_CACHE = None


def _build():
    f32 = mybir.dt.float32
    i32 = mybir.dt.int32
    bf16 = mybir.dt.bfloat16
    Ln = mybir.ActivationFunctionType.Ln
    Copy = mybir.ActivationFunctionType.Copy

    nc = bacc.Bacc("TRN2")
    x_in = nc.declare_dram_parameter("x", [P, FD], f32, isOutput=False)
    g_in = nc.declare_dram_parameter("gt", [P, FD], i32, isOutput=False)
    # one packed output: columns [A | B | C | S], NT each
    out_all = nc.declare_dram_parameter("out_all", [P, 4 * NT], f32, isOutput=True)

    with tile.TileContext(nc) as tc, ExitStack() as ctx:
        xp = ctx.enter_context(tc.tile_pool(name="xp", bufs=5))
        gp = ctx.enter_context(tc.tile_pool(name="gp", bufs=5))
        lp = ctx.enter_context(tc.tile_pool(name="lp", bufs=3))
        jp = ctx.enter_context(tc.tile_pool(name="jp", bufs=1))
        fp = ctx.enter_context(tc.tile_pool(name="fp", bufs=2))
        accp = ctx.enter_context(tc.tile_pool(name="accp", bufs=1))

        acc = accp.tile([P, 4 * NT], f32)

        def col(group, i):
            j = group * NT + i
            return acc[:, j : j + 1]

        off = 0
        for i, tfd in enumerate(TILE_SIZES):
            sl = slice(off, off + tfd)
            off += tfd
            xt = xp.tile([P, tfd], f32, tag="xt")
            gt_t = gp.tile([P, tfd], i32, tag="gt")
            # single SP ring, consumption order: tile i's data lands before
            # tile i+1's, each transfer at full aggregate queue bandwidth
            nc.sync.dma_start(xt[:], x_in[:, sl])
            nc.sync.dma_start(gt_t[:], g_in[:, sl])

            lnx = lp.tile([P, tfd], bf16, tag="lnx")
            ln1 = lp.tile([P, tfd], bf16, tag="ln1")
            nc.scalar.activation(lnx[:], xt[:], Ln)
            nc.scalar.activation(
                ln1[:], xt[:], Ln, bias=1.0, scale=-1.0,
                accum_out=col(2, i),
            )

            junk = jp.tile([P, tfd], bf16, tag="junk")
            nc.vector.scalar_tensor_tensor(
                junk[:], lnx[:], LOG_CLAMP, gt_t[:],
                AluOpType.max, AluOpType.mult,
                accum_out=col(0, i),
            )
            junk2 = jp.tile([P, tfd], bf16, tag="junk")
            nc.vector.scalar_tensor_tensor(
                junk2[:], ln1[:], LOG_CLAMP, gt_t[:],
                AluOpType.max, AluOpType.mult,
                accum_out=col(1, i),
            )

            # s = sum(gt): two add-folds on the idle Pool engine shrink gt
            # 4x, then a short ACT Copy+accum finishes the row sums
            h, q = tfd // 2, tfd // 4
            g2 = fp.tile([P, h], i32, tag="g2")
            nc.gpsimd.tensor_tensor(g2[:], gt_t[:, :h], gt_t[:, h:], AluOpType.add)
            g4 = fp.tile([P, q], i32, tag="g4")
            nc.gpsimd.tensor_tensor(g4[:], g2[:, :q], g2[:, q:], AluOpType.add)
            junk3 = fp.tile([P, q], f32, tag="junk3")
            nc.scalar.activation(junk3[:], g4[:], Copy, accum_out=col(3, i))

        nc.sync.dma_start(out_all[:], acc[:])

    nc.compile()
    return nc


def get_nc():
    global _NC_CACHE
    if _NC_CACHE is None:
        _NC_CACHE = _build()
    return _NC_CACHE


def make_in_maps(x, gt):
    x = np.ascontiguousarray(np.asarray(x, dtype=np.float32).reshape(-1))
    gt = np.ascontiguousarray(np.asarray(gt, dtype=np.int32).reshape(-1))
    assert x.shape == (N_TOTAL,) and gt.shape == (N_TOTAL,)
    in_maps = []
    for c in range(N_CORES):
        sl = slice(c * PER_CORE, (c + 1) * PER_CORE)
        in_maps.append({
            "x": x[sl].reshape(P, FD),
            "gt": gt[sl].reshape(P, FD),
        })
    return in_maps


def combine(results):
    """All-reduce the per-core partial sums and finish the loss formula."""
    A = B = C = S = 0.0
    for r in results:
        o = r["out_all"].astype(np.float64)
        A += o[:, 0 * NT : 1 * NT].sum()
        B += o[:, 1 * NT : 2 * NT].sum()
        C += o[:, 2 * NT : 3 * NT].sum()
        S += o[:, 3 * NT : 4 * NT].sum()
    n = float(N_TOTAL)
    result = -(A / (2.0 * S) + (C - B) / (2.0 * (n - S)))
    return np.array(result, dtype=np.float32)


def kernel(x, gt):
    global LAST_RESULTS
    nc = get_nc()
    in_maps = make_in_maps(x, gt)
    br = run_bass_kernel_spmd(nc, in_maps, list(range(N_CORES)))
    LAST_RESULTS = br
    return combine(br.results)


# revision 9
# speedup vs baseline: 27.5132x; 23.5629x over previous
"""Weighted-BCE loss kernel for Trainium2 (8 NeuronCores, SPMD data-parallel).

Reference math (torch-style BCELoss with class-balancing weights):
    n   = len(x), s = sum(gt)
    w0  = n / (2*(n-s)),  w1 = n / (2*s)
    L1  = max(log(x),     -100)
    L0  = max(log1p(-x),  -100)
    loss = mean( where(gt==0, w0, w1) * -(gt*L1 + (1-gt)*L0) )

The weights depend only on the GLOBAL positive count s, so the loss
decomposes into 4 global sums computed shard-locally:
    A = sum(gt * L1),  B = sum(gt * L0),  C = sum(L0),  s = sum(gt)
    loss = -( A/(2s) + (C-B)/(2(n-s)) )

Each core processes a 1/8 shard laid out [128 partitions, 16384 free].
The kernel is HBM-bound (16.8 MB/core at the ~363 GB/s per-core share of
chip HBM = ~46 us), so the 5 compute passes (2 Ln on ACT, A/B STTs on
DVE, S split ACT/DVE) are balanced to sit a few us under that roofline:
  - All input DMAs stream through the single SP HWDGE ring, interleaved
    x_i, gt_i in consumption order; bufs=5 pools let the ring post
    transfers ahead so the 16 HW DMA queues never gap.
  - ScalarE (ACT): Ln(x) and Ln(1-x) (free affine scale=-1, bias=1);
    the second op's accum_out produces C for free.  ~41us/pass-pair.
  - VectorE (DVE): two fused scalar_tensor_tensor ops, each doing
    clamp(max, -100) + multiply-by-gt + row-reduce in one instruction
    (A and B).  All f32 (mixed bf16/int32 operands measured SLOWER).
  - s = sum(gt): Copy+accum on ACT for most tiles; STT on DVE for two
    early tiles, placed before A/B so they fill DVE's DMA-wait bubbles.
Host gathers the [128, 4*ntiles] partials from all 8 cores and finishes
the (tiny) all-reduce + final scalar arithmetic in float64.
"""

import numpy as np
from contextlib import ExitStack

import concourse.bass as bass
import concourse.bacc as bacc
import concourse.mybir as mybir
import concourse.tile as tile
from concourse.alu_op_type import AluOpType
from concourse.bass_utils import run_bass_kernel_spmd

N_TOTAL = 16777216
N_CORES = 8
PER_CORE = N_TOTAL // N_CORES   # 2097152
P = 128
FD = PER_CORE // P              # 16384 free elements per partition
TILE_SIZES = [2048] * 7 + [1024] * 2
assert sum(TILE_SIZES) == FD
NT = len(TILE_SIZES)
S_ON_DVE = {1, 2}               # ~27% of S columns -> DVE, rest ACT
LOG_CLAMP = -100.0

# Optional instrumentation knobs for a driver script (harness never sets them).
TRACE = False
LAST_RESULTS = None

_NC_CACHE = None


def _build():
    f32 = mybir.dt.float32
    i32 = mybir.dt.int32
    Ln = mybir.ActivationFunctionType.Ln
    Copy = mybir.ActivationFunctionType.Copy

    nc = bacc.Bacc("TRN2")
    x_in = nc.declare_dram_parameter("x", [P, FD], f32, isOutput=False)
    g_in = nc.declare_dram_parameter("gt", [P, FD], i32, isOutput=False)
    # one packed output: columns [A | B | C | S], NT each
    out_all = nc.declare_dram_parameter("out_all", [P, 4 * NT], f32, isOutput=True)

    with tile.TileContext(nc) as tc, ExitStack() as ctx:
        xp = ctx.enter_context(tc.tile_pool(name="xp", bufs=5))
        gp = ctx.enter_context(tc.tile_pool(name="gp", bufs=5))
        lp = ctx.enter_context(tc.tile_pool(name="lp", bufs=2))
        jp = ctx.enter_context(tc.tile_pool(name="jp", bufs=1))
        accp = ctx.enter_context(tc.tile_pool(name="accp", bufs=1))

        acc = accp.tile([P, 4 * NT], f32)

        def col(group, i):
            j = group * NT + i
            return acc[:, j : j + 1]

        off = 0
        for i, tfd in enumerate(TILE_SIZES):
            sl = slice(off, off + tfd)
            off += tfd
            xt = xp.tile([P, tfd], f32, tag="xt")
            gt_t = gp.tile([P, tfd], i32, tag="gt")
            # single SP ring, consumption order: tile i's data lands before
            # tile i+1's, each transfer at full aggregate queue bandwidth
            nc.sync.dma_start(xt[:], x_in[:, sl])
            nc.sync.dma_start(gt_t[:], g_in[:, sl])

            lnx = lp.tile([P, tfd], f32, tag="lnx")
            ln1 = lp.tile([P, tfd], f32, tag="ln1")
            nc.scalar.activation(lnx[:], xt[:], Ln)
            nc.scalar.activation(
                ln1[:], xt[:], Ln, bias=1.0, scale=-1.0,
                accum_out=col(2, i),
            )

            junk3 = jp.tile([P, tfd], f32, tag="junk3")
            if i in S_ON_DVE:
                # needs only gt: placed before A/B to fill DVE's wait on lnx
                nc.vector.scalar_tensor_tensor(
                    junk3[:], gt_t[:], 0.0, gt_t[:],
                    AluOpType.mult, AluOpType.add,
                    accum_out=col(3, i),
                )

            junk = jp.tile([P, tfd], f32, tag="junk")
            nc.vector.scalar_tensor_tensor(
                junk[:], lnx[:], LOG_CLAMP, gt_t[:],
                AluOpType.max, AluOpType.mult,
                accum_out=col(0, i),
            )
            junk2 = jp.tile([P, tfd], f32, tag="junk")
            nc.vector.scalar_tensor_tensor(
                junk2[:], ln1[:], LOG_CLAMP, gt_t[:],
                AluOpType.max, AluOpType.mult,
                accum_out=col(1, i),
            )

            if i not in S_ON_DVE:
                nc.scalar.activation(junk3[:], gt_t[:], Copy, accum_out=col(3, i))

        nc.sync.dma_start(out_all[:], acc[:])

    nc.compile()
    return nc


def get_nc():
    global _NC_CACHE
    if _NC_CACHE is None:
        _NC_CACHE = _build()
    return _NC_CACHE


def make_in_maps(x, gt):
    x = np.ascontiguousarray(np.asarray(x, dtype=np.float32).reshape(-1))
    gt = np.ascontiguousarray(np.asarray(gt, dtype=np.int32).reshape(-1))
    assert x.shape == (N_TOTAL,) and gt.shape == (N_TOTAL,)
    in_maps = []
    for c in range(N_CORES):
        sl = slice(c * PER_CORE, (c + 1) * PER_CORE)
        in_maps.append({
            "x": x[sl].reshape(P, FD),
            "gt": gt[sl].reshape(P, FD),
        })
    return in_maps


def combine(results):
    """All-reduce the per-core partial sums and finish the loss formula."""
    A = B = C = S = 0.0
    for r in results:
        o = r["out_all"].astype(np.float64)
        A += o[:, 0 * NT : 1 * NT].sum()
        B += o[:, 1 * NT : 2 * NT].sum()
        C += o[:, 2 * NT : 3 * NT].sum()
        S += o[:, 3 * NT : 4 * NT].sum()
    n = float(N_TOTAL)
    result = -(A / (2.0 * S) + (C - B) / (2.0 * (n - S)))
    return np.array(result, dtype=np.float32)


def kernel(x, gt):
    global LAST_RESULTS
    nc = get_nc()
    in_maps = make_in_maps(x, gt)
    br = run_bass_kernel_spmd(nc, in_maps, list(range(N_CORES)))
    LAST_RESULTS = br
    return combine(br.results)
